# revision 1
# baseline (speedup 1.0000x reference)
"""TRN2 Bass kernel for nn_DecoderLayer: masked self-attention + cross-attention
+ 2-layer ReLU FFN, data-parallel over the batch dim across 8 NeuronCores.

Contract: kernel(**inputs) takes FULL unsharded inputs (numpy arrays, keyed as
in reference.setup_inputs()) and returns the FULL [8, 2048, 512] fp32 output.

Per-core computation (one batch element b):
    attn1 = softmax(y_b @ y_b.T / sqrt(D) masked) @ y_b
    attn2 = softmax(attn1 @ enc_b.T / sqrt(D)) @ enc_b
    out_b = relu(attn2 @ W1 + b1) @ W2 + b2

The mask is all-ones for this problem's input distribution (spec fill=ones);
the device kernel assumes that and the host wrapper verifies it, falling back
to a numpy reference in the (never exercised) general-mask case.

Kernel strategy ("transposed flash"): activations stay in transposed layout
[d, seq] so probability tiles never need transposing.  Scores are computed in
[k, q] layout (S1 is symmetric; S2 is computed directly transposed), exp on
ACT without max-subtraction (scores bounded by ~25 for these inputs), softmax
denominators via ones-matmul on PE, normalization as a partition-broadcast
multiply on DVE.  Score matmuls for self-attention run in fp8-e4m3 with
perf_mode=DoubleRow (score noise is suppressed by the near-identity softmax,
verified to leave the output error unchanged); the V side and everything
downstream runs in float32r (tf32-rate, 1 cycle/row) to keep the rounding of
values out of the output.  FFN2 uses hT as the stationary operand to flip
back to [q, d] layout, so the output DMA is contiguous.  Work is emitted in
phase sweeps (all blocks of stage 1, then stage 2, then FFN) so the softmax
normalization tail of one block overlaps the matmuls of the next and the PE
never idles long enough for the HAM clock gate to re-throttle.
"""

import numpy as np

B, SD, SE, D = 8, 2048, 1024, 512
P = 128
N_CORES = 8

_CACHE = {}
LAST_RESULT = None


def _install_ntff_shim():
    """Provide antenv.axon_hooks if the image lacks it, so that
    run_bass_kernel_spmd(trace=True) (BASS_TRACE=1) can capture NTFF
    profiles via libaxon's C ABI instead of crashing on the import."""
    import sys
    try:
        import antenv.axon_hooks  # noqa: F401
        return
    except ImportError:
        pass
    import contextlib
    import ctypes
    import types

    _hook = [None]
    so = "/opt/axon/libaxon_pjrt.so"
    try:
        lib = ctypes.CDLL(so)
        if hasattr(lib, "axon_start_nrt_profile"):
            lib.axon_start_nrt_profile.argtypes = [
                ctypes.POINTER(ctypes.c_int64), ctypes.c_size_t]
            lib.axon_start_nrt_profile.restype = ctypes.c_int64
            lib.axon_stop_nrt_profile.argtypes = [ctypes.c_char_p]
            lib.axon_stop_nrt_profile.restype = ctypes.c_int64

            @contextlib.contextmanager
            def hook(output_dir, device_ids):
                import jax
                jax.devices()
                if device_ids:
                    ids = (ctypes.c_int64 * len(device_ids))(*device_ids)
                    rc = lib.axon_start_nrt_profile(ids, len(device_ids))
                else:
                    rc = lib.axon_start_nrt_profile(None, 0)
                if rc != 0:
                    raise RuntimeError(f"axon_start_nrt_profile rc={rc}")
                try:
                    yield
                finally:
                    n = lib.axon_stop_nrt_profile(str(output_dir).encode())
                    if n <= 0:
                        import sys as _s
                        print(f"ntff profile: {n} files written", file=_s.stderr)

            _hook[0] = hook
    except OSError:
        pass

    mod = types.ModuleType("antenv.axon_hooks")
    mod.get_axon_ntff_profile_hook = lambda: _hook[0]

    def _set(h):
        _hook[0] = h

    mod.set_axon_ntff_profile_hook = _set
    import antenv
    antenv.axon_hooks = mod
    sys.modules["antenv.axon_hooks"] = mod


try:
    _install_ntff_shim()
except Exception:
    pass


def _build_module(sd=SD, se=SE, qb=512):
    import concourse.tile as tile
    from concourse import bacc, mybir
    from concourse.masks import make_identity

    FP32 = mybir.dt.float32
    F32R = mybir.dt.float32r
    BF16 = mybir.dt.bfloat16
    Act = mybir.ActivationFunctionType

    DC = D // P           # d chunks (4)
    NQB = sd // qb        # num q blocks
    KT1 = sd // P         # stage-1 k tiles (16)
    KT2 = se // P         # stage-2 k tiles (8)
    QT = qb // P          # q tiles per block
    scale = 1.0 / float(np.sqrt(D))

    nc = bacc.Bacc("TRN2", target_bir_lowering=False, debug=False,
                   enable_asserts=False, num_devices=N_CORES)
    y_d = nc.dram_tensor("y", (sd, D), FP32, kind="ExternalInput").ap()
    enc_d = nc.dram_tensor("enc", (se, D), FP32, kind="ExternalInput").ap()
    w1_d = nc.dram_tensor("w1", (D, D), FP32, kind="ExternalInput").ap()
    b1_d = nc.dram_tensor("b1", (D,), FP32, kind="ExternalInput").ap()
    w2_d = nc.dram_tensor("w2", (D, D), FP32, kind="ExternalInput").ap()
    b2_d = nc.dram_tensor("b2", (D,), FP32, kind="ExternalInput").ap()
    out_d = nc.dram_tensor("out", (sd, D), FP32, kind="ExternalOutput").ap()

    from contextlib import ExitStack

    with tile.TileContext(nc) as tc, \
            tc.tile_pool(name="persist", bufs=1) as persist, \
            tc.tile_pool(name="psum", bufs=1, space="PSUM") as psum, \
            tc.tile_pool(name="psmm", bufs=2, space="PSUM") as psmm, \
            ExitStack() as _late:
        # ==== phase 0: load + precompute layouts ==========================
        with tc.tile_pool(name="staging", bufs=6) as staging:
            ident_f32 = persist.tile([P, P], FP32, tag="ident_f32")
            make_identity(nc, ident_f32[:])

            # y: values in f32r [p, st, d]; queries/keys transposed on PE,
            # cast to fp8-e4m3 for the DoubleRow score matmuls
            F8 = mybir.dt.float8e4
            y_r = persist.tile([P, KT1, D], F32R, tag="y_r")
            yT8 = persist.tile([P, DC, sd], F8, tag="yT8")
            for st in range(KT1):
                stg = staging.tile([P, D], FP32, tag="stg")
                nc.sync.dma_start(stg[:], y_d[st * P:(st + 1) * P, :])
                nc.scalar.copy(y_r[:, st, :], stg[:])
                for dc in range(DC):
                    tp = psmm.tile([P, P], FP32, tag="mm")
                    nc.tensor.transpose(tp[:], stg[:, dc * P:(dc + 1) * P],
                                        ident_f32[:])
                    nc.vector.tensor_copy(yT8[:, dc, st * P:(st + 1) * P], tp[:])

            b1_sb = persist.tile([P, DC], FP32, tag="b1_sb")
            nc.sync.dma_start(b1_sb[:], b1_d.rearrange("(c p) -> p c", p=P))
            b2_sb = persist.tile([P, D], FP32, tag="b2_sb")
            nc.sync.dma_start(b2_sb[:], b2_d.partition_broadcast(P))
            ones_f32 = persist.tile([P, 1], FP32, tag="ones_f32")
            nc.gpsimd.memset(ones_f32[:], 1.0)
            ones_r = persist.tile([P, 1], F32R, tag="ones_r")
            nc.vector.tensor_copy(ones_r[:], ones_f32[:])

            # enc in f32r (rounded on DVE), encT via fp32 transpose
            enc_r = persist.tile([P, KT2, D], F32R, tag="enc_r")
            encT_r = persist.tile([P, DC, se], F32R, tag="encT_r")
            for st in range(KT2):
                stg = staging.tile([P, D], FP32, tag="stg")
                nc.sync.dma_start(stg[:], enc_d[st * P:(st + 1) * P, :])
                nc.scalar.copy(enc_r[:, st, :], stg[:])
                for dc in range(DC):
                    tp = psmm.tile([P, P], FP32, tag="mm")
                    nc.tensor.transpose(tp[:], stg[:, dc * P:(dc + 1) * P],
                                        ident_f32[:])
                    nc.vector.tensor_copy(encT_r[:, dc, st * P:(st + 1) * P], tp[:])

            # weights in f32r
            w1_r = persist.tile([P, DC, D], F32R, tag="w1_r")
            w2_r = persist.tile([P, DC, D], F32R, tag="w2_r")
            for c in range(DC):
                stg = staging.tile([P, D], FP32, tag="stg")
                nc.sync.dma_start(stg[:], w1_d[c * P:(c + 1) * P, :])
                nc.scalar.copy(w1_r[:, c, :], stg[:])
                stg2 = staging.tile([P, D], FP32, tag="stg2")
                nc.sync.dma_start(stg2[:], w2_d[c * P:(c + 1) * P, :])
                nc.scalar.copy(w2_r[:, c, :], stg2[:])

        # work/blk pools open only after staging is released (SBUF budget)
        work = _late.enter_context(tc.tile_pool(name="work", bufs=3))
        blk = _late.enter_context(tc.tile_pool(name="blk", bufs=2))

        # persistent transposed activations (full sweep, enables cross-block
        # overlap of the normalize tail with the next block's matmuls)
        attn1T = persist.tile([P, DC, sd], F32R, tag="attn1T")
        attn2T = persist.tile([P, DC, sd], F32R, tag="attn2T")

        # ==== attention stage (one q block) ===============================
        def attention_stage(kt_n, emit_scores, v_sb, outT_b):
            """outT_b <- normalized attention for one q block, transposed.

            emit_scores(sc, kt): scores matmul group into psum tile sc
            v_sb: [P, kt_n, D] values, natural (lhsT for attn@V, f32r)
            """
            acc = [psum.tile([P, qb], FP32, tag=f"acc{dc}", name=f"acc{dc}")
                   for dc in range(DC)]
            ssum = psum.tile([1, qb], FP32, tag="sum")

            def emit_sc(kt):
                sc = psmm.tile([P, qb], FP32, tag="mm", name="sc")
                emit_scores(sc, kt)
                return sc

            # scores are emitted one kt ahead of the accumulation matmuls so
            # the PE fills the exp (ACT) latency instead of stalling on e[kt]
            sc_next = emit_sc(0)
            for kt in range(kt_n):
                sc_cur, sc_next = sc_next, (emit_sc(kt + 1)
                                            if kt + 1 < kt_n else None)
                e = work.tile([P, qb], mybir.dt.float32r, tag="e", bufs=4)
                nc.scalar.activation(e[:], sc_cur[:], Act.Exp, scale=scale)
                for dc in range(DC):
                    nc.tensor.matmul(
                        acc[dc][:], v_sb[:, kt, dc * P:(dc + 1) * P], e[:],
                        start=(kt == 0), stop=(kt == kt_n - 1),
                    )
                nc.tensor.matmul(
                    ssum[:], ones_r[:], e[:],
                    start=(kt == 0), stop=(kt == kt_n - 1),
                )
            # Copy the accumulators out of PSUM first (releases the banks so
            # the next block's accumulation matmuls start immediately), then
            # normalize from SBUF off the PE critical path.
            accs = [work.tile([P, qb], FP32, tag="accs", bufs=4, name=f"accs{dc}")
                    for dc in range(DC)]
            for dc in range(DC):
                nc.vector.tensor_copy(accs[dc][:], acc[dc][:])
            srow = work.tile([1, qb], FP32, tag="srow", bufs=2)
            nc.vector.tensor_copy(srow[:], ssum[:])
            sbc = work.tile([P, qb], FP32, tag="sbc", bufs=1)
            nc.gpsimd.partition_broadcast(sbc[:], srow[:])
            rbt = work.tile([P, qb], FP32, tag="rbt", bufs=2)
            nc.vector.reciprocal_approx_fast(rbt[:], sbc[:])
            for dc in range(DC):
                nc.vector.tensor_mul(outT_b[:, dc, :], accs[dc][:], rbt[:])

        # ==== phase sweeps ================================================
        DR = mybir.MatmulPerfMode.DoubleRow
        for b in range(NQB):
            qc = slice(b * qb, (b + 1) * qb)

            def s1_scores(sc, kt, qc=qc):
                for dh in range(DC // 2):
                    nc.tensor.matmul(
                        sc[:], yT8[:, 2 * dh:2 * dh + 2, kt * P:(kt + 1) * P],
                        yT8[:, 2 * dh:2 * dh + 2, qc],
                        start=(dh == 0), stop=(dh == DC // 2 - 1),
                        perf_mode=DR,
                    )

            attention_stage(KT1, s1_scores, y_r, attn1T[:, :, qc])
        for b in range(NQB):
            qc = slice(b * qb, (b + 1) * qb)

            def s2_scores(sc, kt, qc=qc):
                for dc in range(DC):
                    nc.tensor.matmul(
                        sc[:], encT_r[:, dc, kt * P:(kt + 1) * P],
                        attn1T[:, dc, qc],
                        start=(dc == 0), stop=(dc == DC - 1),
                    )

            attention_stage(KT2, s2_scores, enc_r, attn2T[:, :, qc])

        for b in range(NQB):
            qc = slice(b * qb, (b + 1) * qb)
            hb = blk.tile([P, DC, qb], F32R, tag="hb")
            for oc in range(DC):
                hp = psmm.tile([P, qb], FP32, tag="mm")
                for ic in range(DC):
                    nc.tensor.matmul(hp[:], w1_r[:, ic, oc * P:(oc + 1) * P],
                                     attn2T[:, ic, qc],
                                     start=(ic == 0), stop=(ic == DC - 1))
                nc.scalar.activation(hb[:, oc, :], hp[:], Act.Relu,
                                     bias=b1_sb[:, oc:oc + 1])
            for qt in range(QT):
                q0 = b * qb + qt * P
                op = psmm.tile([P, D], FP32, tag="mm")
                for ic in range(DC):
                    nc.tensor.matmul(op[:], hb[:, ic, qt * P:(qt + 1) * P],
                                     w2_r[:, ic, :],
                                     start=(ic == 0), stop=(ic == DC - 1))
                ob = work.tile([P, D], FP32, tag="e", bufs=4)
                nc.vector.tensor_add(ob[:], op[:], b2_sb[:])
                nc.sync.dma_start(out_d[q0:q0 + P, :], ob[:])

    nc.compile()
    return nc


def _get_module():
    if "mod" not in _CACHE:
        _CACHE["mod"] = _build_module()
    return _CACHE["mod"]


def _reference_fallback(y, encoder_output, mask, W1, b1, W2, b2):
    """General-mask numpy fallback (not exercised for the spec inputs)."""
    NEG_INF = -1e9

    def sdpa(q, k, v, m):
        s = (q @ k.transpose(0, 2, 1)) / np.float32(np.sqrt(q.shape[-1]))
        if m is not None:
            s = np.where(m, s, NEG_INF)
        s = s - s.max(axis=-1, keepdims=True)
        e = np.exp(s)
        p = e / e.sum(axis=-1, keepdims=True)
        return p @ v

    a1 = sdpa(y, y, y, mask)
    a2 = sdpa(a1, encoder_output, encoder_output, None)
    h = np.maximum(a2 @ W1 + b1, 0.0)
    return (h @ W2 + b2).astype(np.float32)


def kernel(y, encoder_output, mask, W1, b1, W2, b2):
    global LAST_RESULT
    y = np.ascontiguousarray(np.asarray(y, dtype=np.float32))
    enc = np.ascontiguousarray(np.asarray(encoder_output, dtype=np.float32))
    W1 = np.ascontiguousarray(np.asarray(W1, dtype=np.float32))
    b1 = np.ascontiguousarray(np.asarray(b1, dtype=np.float32))
    W2 = np.ascontiguousarray(np.asarray(W2, dtype=np.float32))
    b2 = np.ascontiguousarray(np.asarray(b2, dtype=np.float32))

    if mask is not None and not np.asarray(mask).all():
        return _reference_fallback(y, enc, np.asarray(mask), W1, b1, W2, b2)

    from concourse import bass_utils

    nc = _get_module()
    in_maps = [
        {"y": y[i], "enc": enc[i], "w1": W1, "b1": b1, "w2": W2, "b2": b2}
        for i in range(N_CORES)
    ]
    res = bass_utils.run_bass_kernel_spmd(nc, in_maps, core_ids=list(range(N_CORES)))
    LAST_RESULT = res
    return np.stack([res.results[i]["out"] for i in range(N_CORES)], axis=0)



# revision 8
# speedup vs baseline: 1.1193x; 1.1193x over previous
"""TRN2 Bass kernel for nn_DecoderLayer: masked self-attention + cross-attention
+ 2-layer ReLU FFN, data-parallel over the batch dim across 8 NeuronCores.

Contract: kernel(**inputs) takes FULL unsharded inputs (numpy arrays, keyed as
in reference.setup_inputs()) and returns the FULL [8, 2048, 512] fp32 output.

Per-core computation (one batch element b):
    attn1 = softmax(y_b @ y_b.T / sqrt(D) masked) @ y_b
    attn2 = softmax(attn1 @ enc_b.T / sqrt(D)) @ enc_b
    out_b = relu(attn2 @ W1 + b1) @ W2 + b2

The mask is all-ones for this problem's input distribution (spec fill=ones);
the device kernel assumes that and the host wrapper verifies it, falling back
to a numpy reference in the (never exercised) general-mask case.

Kernel strategy ("transposed flash", v2): activations stay in transposed
layout [d, seq] so probability tiles never need transposing.  Scores are
computed in [k, q] layout, exp on ACT without max-subtraction (scores bounded
for these inputs), softmax denominators accumulated on DVE (esum += e per
k-tile) with a single ones-matmul per block — v1's per-k-tile ones-matmuls
cost a full 512-column PE slot each, 10% of all PE work.

Empirical PE model from the v1 trace: every matmul issues at
~max(N_out x 0.42ns, LDWEIGHTS + 40ns) regardless of dtype; fp8-DoubleRow
does NOT stream columns faster, it only halves instruction count (K=256 per
instruction).  Hence: self-attention scores run fp8-e4m3 DoubleRow (noise
suppressed by the near-identity softmax; verified on the v1 baseline), other
stationary operands are bf16 (a 4-byte f32r stationary LDWEIGHTS is 189ns
and gates the 213ns column stream; bf16 loads hide under it), moving
operands stay f32r for precision.

Input DMA is chunked and interleaved with the first self-attention block:
y chunk c lands -> PE transposes chunk c -> score/acc groups for the k-tiles
of chunk c.  The PE starts ~4us in, instead of idling ~50us behind a
monolithic staging phase (v1's single biggest loss).  Each block's softmax
denominator matmul + normalization is deferred into the NEXT block (after
its first k-tile group) so the PE never waits on the exp/esum tail.
"""

import numpy as np

B, SD, SE, D = 8, 2048, 1024, 512
P = 128
N_CORES = 8

_CACHE = {}
LAST_RESULT = None


def _install_ntff_shim():
    """Provide antenv.axon_hooks if the image lacks it, so that
    run_bass_kernel_spmd(trace=True) (BASS_TRACE=1) can capture NTFF
    profiles via libaxon's C ABI instead of crashing on the import."""
    import sys
    try:
        import antenv.axon_hooks  # noqa: F401
        return
    except ImportError:
        pass
    import contextlib
    import ctypes
    import types

    _hook = [None]
    so = "/opt/axon/libaxon_pjrt.so"
    try:
        lib = ctypes.CDLL(so)
        if hasattr(lib, "axon_start_nrt_profile"):
            lib.axon_start_nrt_profile.argtypes = [
                ctypes.POINTER(ctypes.c_int64), ctypes.c_size_t]
            lib.axon_start_nrt_profile.restype = ctypes.c_int64
            lib.axon_stop_nrt_profile.argtypes = [ctypes.c_char_p]
            lib.axon_stop_nrt_profile.restype = ctypes.c_int64

            @contextlib.contextmanager
            def hook(output_dir, device_ids):
                import jax
                jax.devices()
                if device_ids:
                    ids = (ctypes.c_int64 * len(device_ids))(*device_ids)
                    rc = lib.axon_start_nrt_profile(ids, len(device_ids))
                else:
                    rc = lib.axon_start_nrt_profile(None, 0)
                if rc != 0:
                    raise RuntimeError(f"axon_start_nrt_profile rc={rc}")
                try:
                    yield
                finally:
                    n = lib.axon_stop_nrt_profile(str(output_dir).encode())
                    if n <= 0:
                        import sys as _s
                        print(f"ntff profile: {n} files written", file=_s.stderr)

            _hook[0] = hook
    except OSError:
        pass

    mod = types.ModuleType("antenv.axon_hooks")
    mod.get_axon_ntff_profile_hook = lambda: _hook[0]

    def _set(h):
        _hook[0] = h

    mod.set_axon_ntff_profile_hook = _set
    import antenv
    antenv.axon_hooks = mod
    sys.modules["antenv.axon_hooks"] = mod


try:
    _install_ntff_shim()
except Exception:
    pass


def _build_module(sd=SD, se=SE, qb=512):
    import concourse.tile as tile
    from concourse import bacc, mybir
    from concourse.masks import make_identity

    FP32 = mybir.dt.float32
    F32R = mybir.dt.float32r
    BF16 = mybir.dt.bfloat16
    F8 = mybir.dt.float8e4
    Act = mybir.ActivationFunctionType
    DR = mybir.MatmulPerfMode.DoubleRow

    DC = D // P           # d chunks (4)
    NQB = sd // qb        # num q blocks (4)
    KT1 = sd // P         # stage-1 k tiles (16)
    KT2 = se // P         # stage-2 k tiles (8)
    QT = qb // P          # q tiles per block (4)
    YC = 2                # y tiles per DMA chunk
    NYC = KT1 // YC       # num y chunks (8)
    scale = 1.0 / float(np.sqrt(D))

    nc = bacc.Bacc("TRN2", target_bir_lowering=False, debug=False,
                   enable_asserts=False, num_devices=N_CORES)
    y_d = nc.dram_tensor("y", (sd, D), FP32, kind="ExternalInput").ap()
    enc_d = nc.dram_tensor("enc", (se, D), FP32, kind="ExternalInput").ap()
    w1_d = nc.dram_tensor("w1", (D, D), FP32, kind="ExternalInput").ap()
    b1_d = nc.dram_tensor("b1", (D,), FP32, kind="ExternalInput").ap()
    w2_d = nc.dram_tensor("w2", (D, D), FP32, kind="ExternalInput").ap()
    b2_d = nc.dram_tensor("b2", (D,), FP32, kind="ExternalInput").ap()
    out_d = nc.dram_tensor("out", (sd, D), FP32, kind="ExternalOutput").ap()

    with tile.TileContext(nc) as tc, \
            tc.tile_pool(name="persist", bufs=1) as persist, \
            tc.tile_pool(name="stage", bufs=3) as stage, \
            tc.tile_pool(name="work", bufs=2) as work, \
            tc.tile_pool(name="blk", bufs=2) as blk, \
            tc.tile_pool(name="psum", bufs=1, space="PSUM") as psum, \
            tc.tile_pool(name="psmm", bufs=2, space="PSUM") as psmm, \
            tc.tile_pool(name="pss", bufs=1, space="PSUM") as pss:

        ident = persist.tile([P, P], FP32, tag="ident")
        make_identity(nc, ident[:])
        ones_f32 = persist.tile([P, 1], FP32, tag="ones_f32")
        nc.gpsimd.memset(ones_f32[:], 1.0)
        ones_b = persist.tile([P, 1], BF16, tag="ones_b")
        nc.vector.tensor_copy(ones_b[:], ones_f32[:])

        # persistent device-resident operands
        y_v = persist.tile([P, KT1, D], BF16, tag="y_v")       # V for stage 1
        yT8 = persist.tile([P, DC, sd], F8, tag="yT8")         # Q/K for stage 1
        enc_v = persist.tile([P, KT2, D], BF16, tag="enc_v")   # V for stage 2
        encT = persist.tile([P, DC, se], BF16, tag="encT")     # K^T for stage 2
        w1_sb = persist.tile([P, DC, D], BF16, tag="w1_sb")    # FFN1 stationary
        w2_sb = persist.tile([P, DC, D], BF16, tag="w2_sb")    # FFN2 moving
        b1_sb = persist.tile([P, DC], FP32, tag="b1_sb")
        b2_sb = persist.tile([P, D], FP32, tag="b2_sb")
        attn1T = persist.tile([P, DC, sd], BF16, tag="attn1T")
        attn2T = persist.tile([P, DC, sd], BF16, tag="attn2T")

        # ---- pipelined input staging -------------------------------------
        def load_chunk(src_rows):
            """DMA 2x128 rows of a [*, 512] f32 DRAM tensor into staging."""
            stg = stage.tile([P, YC, D], FP32, tag="stg")
            nc.sync.dma_start(stg[:],
                              src_rows.rearrange("(t p) c -> p t c", p=P))
            return stg

        def prep_y_chunk(c, stg, tp_tag):
            """bf16 V copy + fp8 transposed copy for one y chunk."""
            for t in range(YC):
                st = c * YC + t
                nc.vector.tensor_copy(y_v[:, st, :], stg[:, t, :])
                for dc in range(DC):
                    tp = psmm.tile([P, P], FP32, tag=tp_tag, bufs=1)
                    nc.tensor.transpose(tp[:], stg[:, t, dc * P:(dc + 1) * P],
                                        ident[:])
                    nc.vector.tensor_copy(yT8[:, dc, st * P:(st + 1) * P],
                                          tp[:])

        # ---- deferred block epilogue (denominator + normalize) -----------
        pending = []

        def finish_block(accs, esum, outT_b):
            """Denominator matmul + normalize into outT_b.  Emitted after the
            NEXT block's first k-tile group so the PE never waits on the
            exp/esum tail and the PSUM-release copies."""
            dn = pss.tile([1, qb], FP32, tag="dn")
            nc.tensor.matmul(dn[:], ones_b[:], esum[:], start=True, stop=True)
            rrow = work.tile([1, qb], FP32, tag="rrow", bufs=2)
            nc.vector.reciprocal_approx_fast(rrow[:], dn[:])
            rbc = work.tile([P, qb], FP32, tag="rbc", bufs=2)
            nc.gpsimd.partition_broadcast(rbc[:], rrow[:])
            for dc in range(DC):
                nc.vector.tensor_mul(outT_b[:, dc, :], accs[dc][:], rbc[:])

        def flush():
            while pending:
                finish_block(*pending.pop(0))

        # ---- one attention block -----------------------------------------
        def attn_block(kt_n, emit_scores, v_sb, tag):
            """Scores+exp+attn@V+esum for one q block.  Returns SBUF copies
            of the accumulators and the final esum tile.

            emit_scores(sc, kt): matmul group computing raw scores [P(k), qb]
            v_sb: [P, kt_n, D] values (bf16 stationary for attn @ V)
            """
            acc = [psum.tile([P, qb], FP32, tag=f"acc{dc}", name=f"acc{dc}")
                   for dc in range(DC)]

            def emit_sc(kt):
                sc = psmm.tile([P, qb], FP32, tag="mm", name="sc")
                emit_scores(sc, kt)
                return sc

            lvl = [[] for _ in range(6)]

            def tree_push(t, i=0):
                lvl[i].append(t)
                if len(lvl[i]) == 2:
                    a, b_ = lvl[i]
                    lvl[i].clear()
                    s = work.tile([P, qb], BF16, tag=f"ts{tag}_{i}", bufs=2,
                                  name="tsum")
                    nc.vector.tensor_add(s[:], a[:], b_[:])
                    tree_push(s, i + 1)

            sc_next = emit_sc(0)
            for kt in range(kt_n):
                sc_cur, sc_next = sc_next, (emit_sc(kt + 1)
                                            if kt + 1 < kt_n else None)
                e = work.tile([P, qb], BF16, tag=f"e{tag}", bufs=4)
                nc.scalar.activation(e[:], sc_cur[:], Act.Exp, scale=scale)
                for dc in range(DC):
                    nc.tensor.matmul(
                        acc[dc][:], v_sb[:, kt, dc * P:(dc + 1) * P], e[:],
                        start=(kt == 0), stop=(kt == kt_n - 1),
                    )
                tree_push(e)
                if kt == 0:
                    flush()  # previous block's epilogue
            esum = next(l[0] for l in lvl if l)
            # copy accumulators out of PSUM on the (otherwise idle) GPSIMD
            # engine so the banks free up for the next block's matmuls
            accs = [work.tile([P, qb], FP32, tag=f"as{tag}", bufs=4,
                              name=f"accs{dc}") for dc in range(DC)]
            for dc in range(DC):
                nc.vector.tensor_copy(accs[dc][:], acc[dc][:])
            return accs, esum

        def s1_scores(sc, kt, qc):
            for dh in range(DC // 2):
                nc.tensor.matmul(
                    sc[:], yT8[:, 2 * dh:2 * dh + 2, kt * P:(kt + 1) * P],
                    yT8[:, 2 * dh:2 * dh + 2, qc],
                    start=(dh == 0), stop=(dh == DC // 2 - 1),
                    perf_mode=DR,
                )

        def s2_scores(sc, kt, qc):
            for dc in range(DC):
                nc.tensor.matmul(
                    sc[:], encT[:, dc, kt * P:(kt + 1) * P],
                    attn1T[:, dc, qc],
                    start=(dc == 0), stop=(dc == DC - 1),
                )

        # ==== stage 1 block 0, pipelined with the y input DMA =============
        # k-tile group c covers tiles {2c, 2c+1} and needs y chunk c; the
        # q side (moving fp8) needs chunks 0-1.  DMA runs 2 chunks ahead.
        qc0 = slice(0, qb)
        stg0 = load_chunk(y_d[0:YC * P, :])
        stg1 = load_chunk(y_d[YC * P:2 * YC * P, :])
        pend = load_chunk(y_d[2 * YC * P:3 * YC * P, :])
        prep_y_chunk(0, stg0, "tp")
        prep_y_chunk(1, stg1, "tp")

        acc0 = [psum.tile([P, qb], FP32, tag=f"acc{dc}", name=f"acc{dc}")
                for dc in range(DC)]
        lvl0 = [[] for _ in range(6)]

        def tree_push0(t, i=0):
            lvl0[i].append(t)
            if len(lvl0[i]) == 2:
                a, b_ = lvl0[i]
                lvl0[i].clear()
                s = work.tile([P, qb], BF16, tag=f"ts1_{i}", bufs=2,
                              name="tsum")
                nc.vector.tensor_add(s[:], a[:], b_[:])
                tree_push0(s, i + 1)

        sc_next = psmm.tile([P, qb], FP32, tag="mm", name="sc")
        s1_scores(sc_next, 0, qc0)
        for kt in range(KT1):
            if kt % YC == 1:
                c = (kt + 3) // YC  # next chunk to transpose
                if c < NYC:
                    prep_y_chunk(c, pend, "tp")
                    pend = (load_chunk(y_d[(c+1)*YC*P:(c+2)*YC*P, :])
                            if c + 1 < NYC else None)
            sc_cur = sc_next
            if kt + 1 < KT1:
                sc_next = psmm.tile([P, qb], FP32, tag="mm", name="sc")
                s1_scores(sc_next, kt + 1, qc0)
            else:
                sc_next = None
            e = work.tile([P, qb], BF16, tag="e1", bufs=4)
            nc.scalar.activation(e[:], sc_cur[:], Act.Exp, scale=scale)
            for dc in range(DC):
                nc.tensor.matmul(
                    acc0[dc][:], y_v[:, kt, dc * P:(dc + 1) * P], e[:],
                    start=(kt == 0), stop=(kt == KT1 - 1),
                )
            tree_push0(e)
        accs0 = [work.tile([P, qb], FP32, tag="as1", bufs=4,
                           name=f"accs{dc}") for dc in range(DC)]
        for dc in range(DC):
            nc.vector.tensor_copy(accs0[dc][:], acc0[dc][:])
        esum0 = next(l[0] for l in lvl0 if l)
        pending.append((accs0, esum0, attn1T[:, :, qc0]))

        # remaining input DMA, interleaved with its consumers so that at
        # most 3 staging chunks (bufs=3) are outstanding at a time; the enc
        # transposes go on the PE queue here, right after block 0 (enc has
        # landed by then).
        def prep_enc(c, stg):
            for t in range(YC):
                st = c * YC + t
                nc.vector.tensor_copy(enc_v[:, st, :], stg[:, t, :])
                for dc in range(DC):
                    tp = psmm.tile([P, P], FP32, tag="tp", bufs=1)
                    nc.tensor.transpose(
                        tp[:], stg[:, t, dc * P:(dc + 1) * P], ident[:])
                    nc.vector.tensor_copy(encT[:, dc, st * P:(st + 1) * P],
                                          tp[:])

        def prep_w(w_sb, c, stg):
            for t in range(YC):
                nc.vector.tensor_copy(w_sb[:, c * YC + t, :], stg[:, t, :])

        loads = ([(enc_d, c, prep_enc) for c in range(KT2 // YC)]
                 + [(w1_d, c, lambda c, s: prep_w(w1_sb, c, s))
                    for c in range(DC // YC)]
                 + [(w2_d, c, lambda c, s: prep_w(w2_sb, c, s))
                    for c in range(DC // YC)])
        nc.sync.dma_start(b1_sb[:], b1_d.rearrange("(c p) -> p c", p=P))
        nc.sync.dma_start(b2_sb[:], b2_d.partition_broadcast(P))
        outstanding = []
        for src_d, c, prep in loads:
            if len(outstanding) == 3:
                pc, pstg, pprep = outstanding.pop(0)
                pprep(pc, pstg)
            outstanding.append(
                (c, load_chunk(src_d[c * YC * P:(c + 1) * YC * P, :]), prep))
        for pc, pstg, pprep in outstanding:
            pprep(pc, pstg)

        # ==== stage 1 blocks 1-3 ==========================================
        for b in range(1, NQB):
            qc = slice(b * qb, (b + 1) * qb)
            accs, esum = attn_block(
                KT1, lambda sc, kt, qc=qc: s1_scores(sc, kt, qc), y_v, "1")
            pending.append((accs, esum, attn1T[:, :, qc]))

        # ==== stage 2 ======================================================
        for b in range(NQB):
            qc = slice(b * qb, (b + 1) * qb)
            accs, esum = attn_block(
                KT2, lambda sc, kt, qc=qc: s2_scores(sc, kt, qc), enc_v, "2")
            pending.append((accs, esum, attn2T[:, :, qc]))

        # ==== FFN ==========================================================
        for b in range(NQB):
            qc = slice(b * qb, (b + 1) * qb)
            hb = blk.tile([P, DC, qb], BF16, tag="hb")
            for oc in range(DC):
                hp = psmm.tile([P, qb], FP32, tag="mm")
                for ic in range(DC):
                    nc.tensor.matmul(hp[:], w1_sb[:, ic, oc * P:(oc + 1) * P],
                                     attn2T[:, ic, qc],
                                     start=(ic == 0), stop=(ic == DC - 1))
                nc.scalar.activation(hb[:, oc, :], hp[:], Act.Relu,
                                     bias=b1_sb[:, oc:oc + 1])
                if b == 0 and oc == 0:
                    flush()  # stage 2 block 3 epilogue
            for qt in range(QT):
                q0 = b * qb + qt * P
                op = psmm.tile([P, D], FP32, tag="mm")
                for ic in range(DC):
                    nc.tensor.matmul(op[:], hb[:, ic, qt * P:(qt + 1) * P],
                                     w2_sb[:, ic, :],
                                     start=(ic == 0), stop=(ic == DC - 1))
                ob = work.tile([P, D], FP32, tag="ob", bufs=4)
                nc.vector.tensor_add(ob[:], op[:], b2_sb[:])
                nc.sync.dma_start(out_d[q0:q0 + P, :], ob[:])

    nc.compile()
    return nc


def _get_module():
    if "mod" not in _CACHE:
        _CACHE["mod"] = _build_module()
    return _CACHE["mod"]


def _reference_fallback(y, encoder_output, mask, W1, b1, W2, b2):
    """General-mask numpy fallback (not exercised for the spec inputs)."""
    NEG_INF = -1e9

    def sdpa(q, k, v, m):
        s = (q @ k.transpose(0, 2, 1)) / np.float32(np.sqrt(q.shape[-1]))
        if m is not None:
            s = np.where(m, s, NEG_INF)
        s = s - s.max(axis=-1, keepdims=True)
        e = np.exp(s)
        p = e / e.sum(axis=-1, keepdims=True)
        return p @ v

    a1 = sdpa(y, y, y, mask)
    a2 = sdpa(a1, encoder_output, encoder_output, None)
    h = np.maximum(a2 @ W1 + b1, 0.0)
    return (h @ W2 + b2).astype(np.float32)


def kernel(y, encoder_output, mask, W1, b1, W2, b2):
    global LAST_RESULT
    y = np.ascontiguousarray(np.asarray(y, dtype=np.float32))
    enc = np.ascontiguousarray(np.asarray(encoder_output, dtype=np.float32))
    W1 = np.ascontiguousarray(np.asarray(W1, dtype=np.float32))
    b1 = np.ascontiguousarray(np.asarray(b1, dtype=np.float32))
    W2 = np.ascontiguousarray(np.asarray(W2, dtype=np.float32))
    b2 = np.ascontiguousarray(np.asarray(b2, dtype=np.float32))

    if mask is not None and not np.asarray(mask).all():
        return _reference_fallback(y, enc, np.asarray(mask), W1, b1, W2, b2)

    from concourse import bass_utils

    nc = _get_module()
    in_maps = [
        {"y": y[i], "enc": enc[i], "w1": W1, "b1": b1, "w2": W2, "b2": b2}
        for i in range(N_CORES)
    ]
    res = bass_utils.run_bass_kernel_spmd(nc, in_maps, core_ids=list(range(N_CORES)))
    LAST_RESULT = res
    return np.stack([res.results[i]["out"] for i in range(N_CORES)], axis=0)


# revision 9
# speedup vs baseline: 1.2038x; 1.0754x over previous
"""TRN2 Bass kernel for nn_DecoderLayer: masked self-attention + cross-attention
+ 2-layer ReLU FFN, data-parallel over the batch dim across 8 NeuronCores.

Contract: kernel(**inputs) takes FULL unsharded inputs (numpy arrays, keyed as
in reference.setup_inputs()) and returns the FULL [8, 2048, 512] fp32 output.

Per-core computation (one batch element b):
    attn1 = softmax(y_b @ y_b.T / sqrt(D) masked) @ y_b
    attn2 = softmax(attn1 @ enc_b.T / sqrt(D)) @ enc_b
    out_b = relu(attn2 @ W1 + b1) @ W2 + b2

The mask is all-ones for this problem's input distribution (spec fill=ones);
the device kernel assumes that and the host wrapper verifies it, falling back
to a numpy reference in the (never exercised) general-mask case.

Kernel strategy ("transposed flash", v3): activations stay in transposed
layout [d, seq] so probability tiles never need transposing.  Scores are
computed in [k, q] layout, exp on ACT without max-subtraction (scores bounded
for these inputs), softmax denominators accumulated as a bf16 pairwise tree
on DVE (the per-k-tile ones-matmuls of v1 cost a full 512-column PE slot
each, 10% of all PE work; the final partition reduction is one ones-matmul
per block, and the PE sums partitions exactly in f32 so the bf16 partials
cost ~0.04% denominator error).

Empirical PE model from the v1/v2 traces: every matmul issues at
~max(N_out x 0.42ns, LDWEIGHTS + 40ns) regardless of dtype; fp8-DoubleRow
does NOT stream columns faster, it only halves instruction count (K=256 per
instruction), and a 4-byte f32/f32r stationary LDWEIGHTS (189ns) gates the
213ns column stream.  The walrus verifier also rejects mixing f32/f32r with
other dtypes in one matmul.  Hence: self-attention scores run fp8-e4m3
DoubleRow (noise suppressed by the near-identity softmax), and every other
matmul runs bf16 x bf16 (LDWEIGHTS hides, stream-bound at ~216ns/matmul,
~4e-3 output error vs the 2e-2 gate).

Scheduling: input DMA is chunked and pipelined into the first self-attention
block (PE starts ~12us in, bounded by the engine preamble + first chunk).
Transposes read the persistent bf16 copies, write paired [128, 2x2x128] PSUM
generations, and are drained one generation per k-tile group through a
filler queue so the single PSUM bank never stalls the PE.  Each block's
epilogue is split and deferred into the NEXT block: the DVE tree-tail folds
flush after k-tile 0, the denominator matmul + normalization after k-tile 3,
so the PE never waits on the exp/esum tail or the PSUM-release copies.
"""

import numpy as np

B, SD, SE, D = 8, 2048, 1024, 512
P = 128
N_CORES = 8

_CACHE = {}
LAST_RESULT = None


def _install_ntff_shim():
    """Provide antenv.axon_hooks if the image lacks it, so that
    run_bass_kernel_spmd(trace=True) (BASS_TRACE=1) can capture NTFF
    profiles via libaxon's C ABI instead of crashing on the import."""
    import sys
    try:
        import antenv.axon_hooks  # noqa: F401
        return
    except ImportError:
        pass
    import contextlib
    import ctypes
    import types

    _hook = [None]
    so = "/opt/axon/libaxon_pjrt.so"
    try:
        lib = ctypes.CDLL(so)
        if hasattr(lib, "axon_start_nrt_profile"):
            lib.axon_start_nrt_profile.argtypes = [
                ctypes.POINTER(ctypes.c_int64), ctypes.c_size_t]
            lib.axon_start_nrt_profile.restype = ctypes.c_int64
            lib.axon_stop_nrt_profile.argtypes = [ctypes.c_char_p]
            lib.axon_stop_nrt_profile.restype = ctypes.c_int64

            @contextlib.contextmanager
            def hook(output_dir, device_ids):
                import jax
                jax.devices()
                if device_ids:
                    ids = (ctypes.c_int64 * len(device_ids))(*device_ids)
                    rc = lib.axon_start_nrt_profile(ids, len(device_ids))
                else:
                    rc = lib.axon_start_nrt_profile(None, 0)
                if rc != 0:
                    raise RuntimeError(f"axon_start_nrt_profile rc={rc}")
                try:
                    yield
                finally:
                    n = lib.axon_stop_nrt_profile(str(output_dir).encode())
                    if n <= 0:
                        import sys as _s
                        print(f"ntff profile: {n} files written", file=_s.stderr)

            _hook[0] = hook
    except OSError:
        pass

    mod = types.ModuleType("antenv.axon_hooks")
    mod.get_axon_ntff_profile_hook = lambda: _hook[0]

    def _set(h):
        _hook[0] = h

    mod.set_axon_ntff_profile_hook = _set
    import antenv
    antenv.axon_hooks = mod
    sys.modules["antenv.axon_hooks"] = mod


try:
    _install_ntff_shim()
except Exception:
    pass


def _build_module(sd=SD, se=SE, qb=512):
    import concourse.tile as tile
    from concourse import bacc, mybir
    from concourse.masks import make_identity

    FP32 = mybir.dt.float32
    BF16 = mybir.dt.bfloat16
    F8 = mybir.dt.float8e4
    Act = mybir.ActivationFunctionType
    DR = mybir.MatmulPerfMode.DoubleRow

    DC = D // P           # d chunks (4)
    NQB = sd // qb        # num q blocks (4)
    KT1 = sd // P         # stage-1 k tiles (16)
    KT2 = se // P         # stage-2 k tiles (8)
    QT = qb // P          # q tiles per block (4)
    YC = 2                # 128-row tiles per DMA chunk
    NYC = KT1 // YC       # num y chunks (8)
    scale = 1.0 / float(np.sqrt(D))

    nc = bacc.Bacc("TRN2", target_bir_lowering=False, debug=False,
                   enable_asserts=False, num_devices=N_CORES)
    y_d = nc.dram_tensor("y", (sd, D), FP32, kind="ExternalInput").ap()
    enc_d = nc.dram_tensor("enc", (se, D), FP32, kind="ExternalInput").ap()
    w1_d = nc.dram_tensor("w1", (D, D), FP32, kind="ExternalInput").ap()
    b1_d = nc.dram_tensor("b1", (D,), FP32, kind="ExternalInput").ap()
    w2_d = nc.dram_tensor("w2", (D, D), FP32, kind="ExternalInput").ap()
    b2_d = nc.dram_tensor("b2", (D,), FP32, kind="ExternalInput").ap()
    out_d = nc.dram_tensor("out", (sd, D), FP32, kind="ExternalOutput").ap()

    with tile.TileContext(nc) as tc, \
            tc.tile_pool(name="persist", bufs=1) as persist, \
            tc.tile_pool(name="stage", bufs=3) as stage, \
            tc.tile_pool(name="work", bufs=2) as work, \
            tc.tile_pool(name="blk", bufs=2) as blk, \
            tc.tile_pool(name="psum", bufs=1, space="PSUM") as psum, \
            tc.tile_pool(name="psmm", bufs=2, space="PSUM") as psmm, \
            tc.tile_pool(name="pss", bufs=1, space="PSUM") as pss:

        ident_b = persist.tile([P, P], BF16, tag="ident_b")
        make_identity(nc, ident_b[:])
        ones_f32 = persist.tile([P, 1], FP32, tag="ones_f32")
        nc.gpsimd.memset(ones_f32[:], 1.0)
        ones_b = persist.tile([P, 1], BF16, tag="ones_b")
        nc.vector.tensor_copy(ones_b[:], ones_f32[:])

        # persistent device-resident operands (bf16 except fp8 score copies)
        y_v = persist.tile([P, KT1, D], BF16, tag="y_v")       # V for stage 1
        yT8 = persist.tile([P, DC, sd], F8, tag="yT8")         # Q/K for stage 1
        enc_v = persist.tile([P, KT2, D], BF16, tag="enc_v")   # V for stage 2
        encT = persist.tile([P, DC, se], BF16, tag="encT")     # K^T for stage 2
        w1_sb = persist.tile([P, DC, D], BF16, tag="w1_sb")    # FFN1 stationary
        w2_sb = persist.tile([P, DC, D], BF16, tag="w2_sb")    # FFN2 moving
        b1_sb = persist.tile([P, DC], FP32, tag="b1_sb")
        b2_sb = persist.tile([P, D], FP32, tag="b2_sb")
        attn1T = persist.tile([P, DC, sd], BF16, tag="attn1T")
        attn2T = persist.tile([P, DC, sd], BF16, tag="attn2T")

        # ---- pipelined input staging -------------------------------------
        def load_chunk(src_rows):
            """DMA 2x128 rows of a [*, 512] f32 DRAM tensor into staging."""
            stg = stage.tile([P, YC, D], FP32, tag="stg")
            nc.sync.dma_start(stg[:],
                              src_rows.rearrange("(t p) c -> p t c", p=P))
            return stg

        # filler queue: each entry emits one PSUM transpose generation (4
        # transposes + 2 batched copies); drained one per k-tile group so
        # the single tp PSUM bank never stalls the PE.
        fillers = []

        def drain_filler():
            if fillers:
                fillers.pop(0)()

        def t_gen(src_v, dstT, st0, h):
            """Transpose dc pair (2h, 2h+1) of tiles (st0, st0+1) into dstT."""
            tp = psmm.tile([P, 2, YC, P], BF16, tag="tp", bufs=1, name="tp")
            for i in range(2):
                dc = 2 * h + i
                for t in range(YC):
                    nc.tensor.transpose(
                        tp[:, i, t, :],
                        src_v[:, st0 + t, dc * P:(dc + 1) * P], ident_b[:])
                nc.vector.tensor_copy(dstT[:, dc, st0 * P:(st0 + YC) * P],
                                      tp[:, i, :, :])

        # ---- deferred block epilogue --------------------------------------
        # stage_a (after next block's k-tile 0): DVE folds of the esum tree
        # leftovers; stage_b (after k-tile 3): denominator matmul + normalize.
        pending = []

        def stage_a(ent):
            accs, leftovers, outT_b = ent
            s = leftovers[0]
            for t in leftovers[1:]:
                f = work.tile([P, qb], BF16, tag="fold", bufs=2, name="fold")
                nc.vector.tensor_add(f[:], s[:], t[:])
                s = f
            return (accs, s, outT_b)

        def stage_b(ent):
            accs, esum, outT_b = ent
            dn = pss.tile([1, qb], FP32, tag="dn")
            nc.tensor.matmul(dn[:], ones_b[:], esum[:], start=True, stop=True)
            rrow = work.tile([1, qb], FP32, tag="rrow", bufs=2)
            nc.vector.reciprocal_approx_fast(rrow[:], dn[:])
            rbc = work.tile([P, qb], FP32, tag="rbc", bufs=2)
            nc.gpsimd.partition_broadcast(rbc[:], rrow[:])
            for dc in range(DC):
                nc.vector.tensor_mul(outT_b[:, dc, :], accs[dc][:], rbc[:])

        def epilogue_hooks(kt):
            if kt == 0 and pending:
                pending[0] = stage_a(pending[0])
            elif kt == 3 and pending:
                stage_b(pending.pop(0))
            drain_filler()

        # ---- one attention block -------------------------------------------
        def attn_block(kt_n, emit_scores, v_sb, tag):
            """Scores+exp+attn@V+esum-tree for one q block.  Returns SBUF
            copies of the accumulators and the un-folded tree leftovers."""
            acc = [psum.tile([P, qb], FP32, tag=f"acc{dc}", name=f"acc{dc}")
                   for dc in range(DC)]
            lvl = [[] for _ in range(6)]

            def tree_push(t, i=0):
                lvl[i].append(t)
                if len(lvl[i]) == 2:
                    a, b_ = lvl[i]
                    lvl[i].clear()
                    s = work.tile([P, qb], BF16, tag=f"ts{tag}_{i}", bufs=2,
                                  name="tsum")
                    nc.vector.tensor_add(s[:], a[:], b_[:])
                    tree_push(s, i + 1)

            def emit_sc(kt):
                sc = psmm.tile([P, qb], FP32, tag="mm", name="sc")
                emit_scores(sc, kt)
                return sc

            leftovers = []
            sc_next = emit_sc(0)
            for kt in range(kt_n):
                sc_cur, sc_next = sc_next, (emit_sc(kt + 1)
                                            if kt + 1 < kt_n else None)
                e = work.tile([P, qb], BF16, tag=f"e{tag}", bufs=4)
                nc.scalar.activation(e[:], sc_cur[:], Act.Exp, scale=scale)
                for dc in range(DC):
                    nc.tensor.matmul(
                        acc[dc][:], v_sb[:, kt, dc * P:(dc + 1) * P], e[:],
                        start=(kt == 0), stop=(kt == kt_n - 1),
                    )
                if kt < kt_n - 1:
                    tree_push(e)
                else:
                    leftovers = [e] + [l[0] for l in lvl if l]
                epilogue_hooks(kt)
            accs = [work.tile([P, qb], FP32, tag=f"as{tag}", bufs=4,
                              name=f"accs{dc}") for dc in range(DC)]
            for dc in range(DC):
                nc.vector.tensor_copy(accs[dc][:], acc[dc][:])
            return accs, leftovers

        def s1_scores(sc, kt, qc):
            for dh in range(DC // 2):
                nc.tensor.matmul(
                    sc[:], yT8[:, 2 * dh:2 * dh + 2, kt * P:(kt + 1) * P],
                    yT8[:, 2 * dh:2 * dh + 2, qc],
                    start=(dh == 0), stop=(dh == DC // 2 - 1),
                    perf_mode=DR,
                )

        def s2_scores(sc, kt, qc):
            for dc in range(DC):
                nc.tensor.matmul(
                    sc[:], encT[:, dc, kt * P:(kt + 1) * P],
                    attn1T[:, dc, qc],
                    start=(dc == 0), stop=(dc == DC - 1),
                )

        # ==== stage 1 block 0, pipelined with the y input DMA ==============
        # k-tile group {2c, 2c+1} needs y chunk c; the q side (moving fp8)
        # needs chunks 0-1 up front.  DMA runs ~2 chunks ahead of the PE.
        qc0 = slice(0, qb)
        stg0 = load_chunk(y_d[0:YC * P, :])
        stg1 = load_chunk(y_d[YC * P:2 * YC * P, :])
        pend = load_chunk(y_d[2 * YC * P:3 * YC * P, :])
        nc.vector.tensor_copy(y_v[:, 0:YC, :], stg0[:])
        nc.vector.tensor_copy(y_v[:, YC:2 * YC, :], stg1[:])
        for c in range(2):
            for h in range(2):
                t_gen(y_v, yT8, c * YC, h)

        acc0 = [psum.tile([P, qb], FP32, tag=f"acc{dc}", name=f"acc{dc}")
                for dc in range(DC)]
        lvl0 = [[] for _ in range(6)]

        def tree_push0(t, i=0):
            lvl0[i].append(t)
            if len(lvl0[i]) == 2:
                a, b_ = lvl0[i]
                lvl0[i].clear()
                s = work.tile([P, qb], BF16, tag=f"ts1_{i}", bufs=2,
                              name="tsum")
                nc.vector.tensor_add(s[:], a[:], b_[:])
                tree_push0(s, i + 1)

        leftovers0 = []
        sc_next = psmm.tile([P, qb], FP32, tag="mm", name="sc")
        s1_scores(sc_next, 0, qc0)
        for kt in range(KT1):
            if kt % YC == 1:
                c = (kt + 3) // YC  # next chunk to prepare
                if c < NYC:
                    nc.vector.tensor_copy(y_v[:, c * YC:(c + 1) * YC, :],
                                          pend[:])
                    fillers.append(lambda c=c: t_gen(y_v, yT8, c * YC, 0))
                    fillers.append(lambda c=c: t_gen(y_v, yT8, c * YC, 1))
                    pend = (load_chunk(y_d[(c+1)*YC*P:(c+2)*YC*P, :])
                            if c + 1 < NYC else None)
            sc_cur = sc_next
            if kt + 1 < KT1:
                sc_next = psmm.tile([P, qb], FP32, tag="mm", name="sc")
                s1_scores(sc_next, kt + 1, qc0)
            else:
                sc_next = None
            e = work.tile([P, qb], BF16, tag="e1", bufs=4)
            nc.scalar.activation(e[:], sc_cur[:], Act.Exp, scale=scale)
            for dc in range(DC):
                nc.tensor.matmul(
                    acc0[dc][:], y_v[:, kt, dc * P:(dc + 1) * P], e[:],
                    start=(kt == 0), stop=(kt == KT1 - 1),
                )
            if kt < KT1 - 1:
                tree_push0(e)
            else:
                leftovers0 = [e] + [l[0] for l in lvl0 if l]
            drain_filler()
        accs0 = [work.tile([P, qb], FP32, tag="as1", bufs=4,
                           name=f"accs{dc}") for dc in range(DC)]
        for dc in range(DC):
            nc.vector.tensor_copy(accs0[dc][:], acc0[dc][:])
        pending.append((accs0, leftovers0, attn1T[:, :, qc0]))

        # remaining inputs: DMA + immediate bf16 casts (releases staging);
        # the enc transposes become fillers drained inside stage-1 block 1.
        for c in range(KT2 // YC):
            stg = load_chunk(enc_d[c * YC * P:(c + 1) * YC * P, :])
            nc.vector.tensor_copy(enc_v[:, c * YC:(c + 1) * YC, :], stg[:])
            fillers.append(lambda c=c: t_gen(enc_v, encT, c * YC, 0))
            fillers.append(lambda c=c: t_gen(enc_v, encT, c * YC, 1))
        for w_sb, w_src in ((w1_sb, w1_d), (w2_sb, w2_d)):
            for c in range(DC // YC):
                stg = load_chunk(w_src[c * YC * P:(c + 1) * YC * P, :])
                nc.vector.tensor_copy(w_sb[:, c * YC:(c + 1) * YC, :], stg[:])
        nc.sync.dma_start(b1_sb[:], b1_d.rearrange("(c p) -> p c", p=P))
        nc.sync.dma_start(b2_sb[:], b2_d.partition_broadcast(P))

        # ==== stage 1 blocks 1-3 ===========================================
        for b in range(1, NQB):
            qc = slice(b * qb, (b + 1) * qb)
            accs, leftovers = attn_block(
                KT1, lambda sc, kt, qc=qc: s1_scores(sc, kt, qc), y_v, "1")
            pending.append((accs, leftovers, attn1T[:, :, qc]))

        # ==== stage 2 ======================================================
        for b in range(NQB):
            qc = slice(b * qb, (b + 1) * qb)
            accs, leftovers = attn_block(
                KT2, lambda sc, kt, qc=qc: s2_scores(sc, kt, qc), enc_v, "2")
            pending.append((accs, leftovers, attn2T[:, :, qc]))

        # ==== FFN ==========================================================
        for b in range(NQB):
            qc = slice(b * qb, (b + 1) * qb)
            hb = blk.tile([P, DC, qb], BF16, tag="hb")
            for oc in range(DC):
                hp = psmm.tile([P, qb], FP32, tag="mm")
                for ic in range(DC):
                    nc.tensor.matmul(hp[:], w1_sb[:, ic, oc * P:(oc + 1) * P],
                                     attn2T[:, ic, qc],
                                     start=(ic == 0), stop=(ic == DC - 1))
                nc.scalar.activation(hb[:, oc, :], hp[:], Act.Relu,
                                     bias=b1_sb[:, oc:oc + 1])
                if b == 0 and oc == 0 and pending:
                    pending[0] = stage_a(pending[0])
                if b == 0 and oc == 3 and pending:
                    stage_b(pending.pop(0))
            for qt in range(QT):
                q0 = b * qb + qt * P
                op = psmm.tile([P, D], FP32, tag="mm")
                for ic in range(DC):
                    nc.tensor.matmul(op[:], hb[:, ic, qt * P:(qt + 1) * P],
                                     w2_sb[:, ic, :],
                                     start=(ic == 0), stop=(ic == DC - 1))
                ob = work.tile([P, D], FP32, tag="ob", bufs=4)
                nc.vector.tensor_add(ob[:], op[:], b2_sb[:])
                nc.sync.dma_start(out_d[q0:q0 + P, :], ob[:])

    nc.compile()
    return nc


def _get_module():
    if "mod" not in _CACHE:
        _CACHE["mod"] = _build_module()
    return _CACHE["mod"]


def _reference_fallback(y, encoder_output, mask, W1, b1, W2, b2):
    """General-mask numpy fallback (not exercised for the spec inputs)."""
    NEG_INF = -1e9

    def sdpa(q, k, v, m):
        s = (q @ k.transpose(0, 2, 1)) / np.float32(np.sqrt(q.shape[-1]))
        if m is not None:
            s = np.where(m, s, NEG_INF)
        s = s - s.max(axis=-1, keepdims=True)
        e = np.exp(s)
        p = e / e.sum(axis=-1, keepdims=True)
        return p @ v

    a1 = sdpa(y, y, y, mask)
    a2 = sdpa(a1, encoder_output, encoder_output, None)
    h = np.maximum(a2 @ W1 + b1, 0.0)
    return (h @ W2 + b2).astype(np.float32)


def kernel(y, encoder_output, mask, W1, b1, W2, b2):
    global LAST_RESULT
    y = np.ascontiguousarray(np.asarray(y, dtype=np.float32))
    enc = np.ascontiguousarray(np.asarray(encoder_output, dtype=np.float32))
    W1 = np.ascontiguousarray(np.asarray(W1, dtype=np.float32))
    b1 = np.ascontiguousarray(np.asarray(b1, dtype=np.float32))
    W2 = np.ascontiguousarray(np.asarray(W2, dtype=np.float32))
    b2 = np.ascontiguousarray(np.asarray(b2, dtype=np.float32))

    if mask is not None and not np.asarray(mask).all():
        return _reference_fallback(y, enc, np.asarray(mask), W1, b1, W2, b2)

    from concourse import bass_utils

    nc = _get_module()
    in_maps = [
        {"y": y[i], "enc": enc[i], "w1": W1, "b1": b1, "w2": W2, "b2": b2}
        for i in range(N_CORES)
    ]
    res = bass_utils.run_bass_kernel_spmd(nc, in_maps, core_ids=list(range(N_CORES)))
    LAST_RESULT = res
    return np.stack([res.results[i]["out"] for i in range(N_CORES)], axis=0)
